# revision 18
# baseline (speedup 1.0000x reference)
"""Trainium2 Bass kernel for nn_MultiHeadAttention_59227599012491.

Reference computation (per batch b):
    xf = x[b].reshape(S, 256)
    q  = softplus(xf @ Wq.T + bq);  k = softplus(xf @ Wk.T + bk)
    v  = xf @ Wv.T + bv
    out = ((q @ k.T) @ v) @ Wo.T + bo          (no softmax!)

No softmax -> associativity: out = q @ M + bo with
    G = k.T @ v   [256,256],   M = G @ Wo.T   [256,256]
so the S x S score matrix never exists. Sharding: B=4 batches x 2
query-halves -> 8 cores, no collectives (an NRT AllReduce of M was
measured at ~17 us fixed latency -- more than the whole dedup saves, so
k/v/G/M are recomputed by both cores of a pair; queries + output rows
are split).

Per-core pipeline (all matmuls fp16, PE computes out = lhsT.T @ rhs):
    kv loop (4 groups of 8 seq tiles): ps = x_tile @ [WkT|WvT]; DVE
        adds bk to the k plane, GpSimd adds bv to the v plane (psum ->
        fp16); batched ACT Exp+Ln softplus over each group's k planes;
        GT[d,e] += v_tile^T k_tile accumulated in PSUM across all 32
        tiles. One qT chunk is interleaved after each group so the ACT
        engine's softplus backlog hides under PE work.
    qT [e,s] = softplus(Wq x^T + bq): per-partition bias fused into the
        ACT Exp pass straight out of PSUM.
    M = G @ WoT (tiny), then outT [do,s] = M^T q^T + bo: transposed
        output so bo is per-partition (DVE tensor_scalar_add) and the
        fp16 DRAM dump is contiguous 2 KB runs per partition; the host
        un-transposes and casts back to fp32.

DMA: every input DMA moves 2 KB descriptors (host-packed layouts); each
DMA instruction occupies one HW queue (~60 GB/s at 2 KB descriptors),
so the load is split into pieces across three issuing engines (sync +
scalar HWDGE, gpsimd SWDGE) for queue parallelism, with the
first-needed pieces (Wkv, x cols 0:1024) split by partition halves to
land earliest. Output: 4 chunks of [128,1024] fp16, each written as two
[64,1024] pieces on alternating queues so the final chunk drains fast.

The activation-table pass is steered to `natural_log_exp_and_others`
(the only set holding Exp AND Ln) so the ACT engine loads its PWP
table exactly once.
"""

import numpy as np

S = 4096
SQ = 2048  # query rows per core
D = 256
P = 128
IT = D // P  # 2 contraction tiles over d
NS = S // P  # 32 sequence tiles
GRP = 8  # kv tiles per softplus batch
NG = NS // GRP
N_CORES = 8

_CACHE = {}


def _patched_act_tables(orig_fn):
    def patched(arch):
        tabs = orig_fn(arch)
        return {
            name: (s if name == "natural_log_exp_and_others" else set())
            for name, s in tabs.items()
        }

    return patched


def _build_nc():
    import concourse.bacc as bacc
    import concourse.mybir as mybir
    import concourse.tile as tile

    FP = mybir.dt.float32
    FR = mybir.dt.float16
    AF = mybir.ActivationFunctionType
    ADD = mybir.AluOpType.add

    nc = bacc.Bacc("TRN2", target_bir_lowering=False, debug=False, num_devices=1)

    # x pieces: [8, 128, 1024], piece it*4+cc = x^T[it-block, cc*1024:...]
    xp_d = nc.declare_dram_parameter("xp", [2 * 4, P, 1024], FR, isOutput=False)
    wkv_d = nc.declare_dram_parameter("wkvp", [P, 2 * 512], FR, isOutput=False)
    wqo_d = nc.declare_dram_parameter("wqop", [P, 2 * 512], FR, isOutput=False)
    bias_d = nc.declare_dram_parameter("biasp", [P, 4], FP, isOutput=False)
    bkv_d = nc.declare_dram_parameter("bkv", [1, 2 * D], FP, isOutput=False)
    outp_d = nc.declare_dram_parameter("outp", [P, 2 * SQ], FR, isOutput=True)

    def mm(psum, lhsT, rhs, start, stop):
        nc.tensor.matmul(psum, lhsT, rhs, start=start, stop=stop)

    with tile.TileContext(nc) as tc:
        with (
            tc.tile_pool(name="w", bufs=1) as wpool,
            tc.tile_pool(name="big", bufs=1) as big,
            tc.tile_pool(name="tmp", bufs=4) as tpool,
            tc.tile_pool(name="psKV", bufs=3, space="PSUM") as psKV,
            tc.tile_pool(name="psG", bufs=1, space="PSUM") as psG,
            tc.tile_pool(name="psQ", bufs=3, space="PSUM") as psQ,
        ):
            # SBUF weight layout: cols [wkv 512 | wq 256 | wo 256] per it
            w_sb = wpool.tile([P, IT, 1024], FR, tag="w")
            xbT_sb = big.tile([P, IT, S], FR, tag="xbT")
            bias_sb = wpool.tile([P, 4], FP, tag="bias")
            bkv_bc = wpool.tile([P, 2 * D], FP, tag="bkv")
            kv_sb = big.tile([P, 2, NS, D], FR, tag="kv")
            qT_sb = big.tile([P, IT, SQ], FR, tag="qT")
            GT_sb = wpool.tile([P, IT, D], FR, tag="GT")
            M_sb = wpool.tile([P, IT, D], FR, tag="M")
            outT_sb = big.tile([P, IT, SQ], FR, tag="outT")

            # --- input DMAs: one HW queue per instruction, spread over
            # sync/scalar (HWDGE) + gpsimd (SWDGE); first-needed first ---
            nc.sync.dma_start(
                w_sb[0:64, :, 0:512],
                wkv_d.ap()[0:64, :].rearrange("p (i w) -> p i w", i=IT),
            )
            nc.scalar.dma_start(
                w_sb[64:P, :, 0:512],
                wkv_d.ap()[64:P, :].rearrange("p (i w) -> p i w", i=IT),
            )
            nc.gpsimd.dma_start(
                bkv_bc[:, :], bkv_d.ap()[0:1, :].broadcast_to([P, 2 * D])
            )
            nc.gpsimd.dma_start(bias_sb[:, :], bias_d.ap()[:, :])

            def xpiece(eng, it, cc, hp=slice(0, P)):
                eng.dma_start(
                    xbT_sb[hp, it, cc * 1024 : (cc + 1) * 1024],
                    xp_d.ap()[it * 4 + cc, hp, :],
                )

            # first column chunk split into partition halves to land first
            xpiece(nc.sync, 0, 0, slice(0, 64))
            xpiece(nc.scalar, 0, 0, slice(64, P))
            xpiece(nc.sync, 1, 0, slice(0, 64))
            xpiece(nc.scalar, 1, 0, slice(64, P))
            xpiece(nc.sync, 0, 1)
            xpiece(nc.gpsimd, 1, 1)
            xpiece(nc.scalar, 0, 2)
            xpiece(nc.sync, 1, 2)
            xpiece(nc.scalar, 0, 3)
            xpiece(nc.sync, 1, 3)
            nc.gpsimd.dma_start(
                w_sb[:, :, 512:1024],
                wqo_d.ap()[:, :].rearrange("p (i w) -> p i w", i=IT),
            )

            wkv = w_sb[:, :, 0:512]

            # --- kv = x [WkT | WvT] + [bk | bv]; softplus k in 4-tile runs ---
            for t in range(NS):
                ts = slice(t * P, (t + 1) * P)
                ps = psKV.tile([P, 2 * D], FP, tag="psKV")
                for it in range(IT):
                    mm(ps[:, :], xbT_sb[:, it, ts], wkv[:, it, :], it == 0, it == IT - 1)
                nc.vector.tensor_tensor(
                    kv_sb[:, :, t, :],
                    ps[:, :].rearrange("p (j d) -> p j d", j=2),
                    bkv_bc[:, :].rearrange("p (j d) -> p j d", j=2),
                    op=ADD,
                )
                if t % 4 == 3:
                    tt = slice(t - 3, t + 1)
                    tmpk = tpool.tile([P, 4, D], FP, tag="tmpk")
                    nc.scalar.activation(tmpk[:, :, :], kv_sb[:, 0, tt, :], AF.Exp)
                    nc.scalar.activation(kv_sb[:, 0, tt, :], tmpk[:, :, :], AF.Ln, bias=1.0)

            # --- qT = softplus(Wq x^T + bq), [e, s]; bias fused in Exp ---
            for dt in range(IT):
                for blk in range(SQ // 512):
                    ss = slice(blk * 512, (blk + 1) * 512)
                    ps = psQ.tile([P, 512], FP, tag="psQ")
                    for it in range(IT):
                        mm(
                            ps[:, :],
                            w_sb[:, it, 512 + dt * P : 512 + (dt + 1) * P],
                            xbT_sb[:, it, ss],
                            it == 0,
                            it == IT - 1,
                        )
                    tmpq = tpool.tile([P, 512], FP, tag="tmpq")
                    nc.scalar.activation(
                        tmpq[:, :], ps[:, :], AF.Exp, bias=bias_sb[:, dt : dt + 1]
                    )
                    nc.scalar.activation(qT_sb[:, dt, ss], tmpq[:, :], AF.Ln, bias=1.0)

            # --- GT[d, e] = sum_s v[s, d] k[s, e] (pure PE streaming) ---
            GTps = []
            for dt in range(IT):
                gt = psG.tile([P, D], FP, tag=f"psG{dt}", name=f"GTps{dt}")
                GTps.append(gt)
            for dt in range(IT):
                vs = slice(dt * P, (dt + 1) * P)
                for t in range(NS):
                    mm(GTps[dt], kv_sb[:, 1, t, vs], kv_sb[:, 0, t, :], t == 0, t == NS - 1)
                nc.vector.tensor_copy(GT_sb[:, dt, :], GTps[dt][:, :])

            # --- M = GT^T @ WoT ---
            for et in range(IT):
                ps = psQ.tile([P, 512], FP, tag="psQ")
                for dt in range(IT):
                    mm(
                        ps[:, 0:D],
                        GT_sb[:, dt, et * P : (et + 1) * P],
                        w_sb[:, dt, 768:1024],
                        dt == 0,
                        dt == IT - 1,
                    )
                nc.vector.tensor_copy(M_sb[:, et, :], ps[:, 0:D])

            # --- outT[do, s] = M^T q^T + bo (per-partition bias, fp16) ---
            for dot in range(IT):
                for blk in range(SQ // 512):
                    ss = slice(blk * 512, (blk + 1) * 512)
                    ps = psQ.tile([P, 512], FP, tag="psQ")
                    for et in range(IT):
                        mm(
                            ps[:, :],
                            M_sb[:, et, dot * P : (dot + 1) * P],
                            qT_sb[:, et, ss],
                            et == 0,
                            et == IT - 1,
                        )
                    nc.vector.tensor_scalar_add(
                        outT_sb[:, dot, ss], ps[:, :], bias_sb[:, 2 + dot : 3 + dot]
                    )
                    if blk % 2 == 1:
                        off = dot * SQ + (blk - 1) * 512
                        src = outT_sb[:, dot, (blk - 1) * 512 : (blk + 1) * 512]
                        last = dot == IT - 1 and blk == SQ // 512 - 1
                        if last:
                            engs = [nc.sync, nc.gpsimd, nc.scalar, nc.sync]
                            for qi in range(4):
                                hp = slice(qi * 32, (qi + 1) * 32)
                                engs[qi].dma_start(
                                    outp_d.ap()[hp, off : off + 1024], src[hp, :]
                                )
                        else:
                            nc.sync.dma_start(outp_d.ap()[:, off : off + 1024], src)

    import concourse.hw_specs as hw_specs

    orig = bacc.get_activation_tables
    bacc.get_activation_tables = _patched_act_tables(hw_specs.get_activation_tables)
    try:
        nc.compile()
    finally:
        bacc.get_activation_tables = orig
    return nc


def _get_nc():
    nc = _CACHE.get("nc")
    if nc is None:
        nc = _build_nc()
        _CACHE["nc"] = nc
    return nc


def make_in_maps(x, Wq, bq, Wk, bk, Wv, bv, Wo, bo):
    B = x.shape[0]
    f16 = np.float16
    xf = np.asarray(x, dtype=np.float32).reshape(B, S, D)
    xfT = np.ascontiguousarray(xf.transpose(0, 2, 1).astype(f16))  # [B, 256, 4096]

    def pack_it(wT):  # [256, C] -> [128, 2*C] with it-blocks side by side
        C = wT.shape[1]
        return np.ascontiguousarray(
            wT.reshape(IT, P, C).transpose(1, 0, 2).reshape(P, IT * C)
        )

    wkvp = pack_it(np.hstack([np.asarray(Wk, f16).T, np.asarray(Wv, f16).T]))
    wqop = pack_it(np.hstack([np.asarray(Wq, f16).T, np.asarray(Wo, f16).T]))
    biasp = np.stack(
        [
            np.asarray(bq, np.float32)[0:P],
            np.asarray(bq, np.float32)[P:D],
            np.asarray(bo, np.float32)[0:P],
            np.asarray(bo, np.float32)[P:D],
        ],
        axis=1,
    )
    shared = {
        "wkvp": wkvp,
        "wqop": wqop,
        "biasp": np.ascontiguousarray(biasp),
        "bkv": np.concatenate(
            [np.asarray(bk, np.float32), np.asarray(bv, np.float32)]
        ).reshape(1, 2 * D),
    }
    in_maps = []
    for c in range(N_CORES):
        b, h = divmod(c, 2)
        xT = xfT[b]
        if h == 1:
            xT = np.concatenate([xT[:, SQ:], xT[:, :SQ]], axis=1)
        # pieces [it*4+cc] = [128, 1024]
        xpc = np.ascontiguousarray(
            xT.reshape(IT, P, 4, 1024).transpose(0, 2, 1, 3).reshape(8, P, 1024)
        )
        in_maps.append({"xp": xpc, **shared})
    return in_maps


def assemble_out(results, x_shape):
    B, S_, H, W = x_shape
    out = np.empty((B, S_, D), np.float32)
    for c in range(N_CORES):
        b, h = divmod(c, 2)
        outp = results[c]["outp"]  # [128, 2*SQ] fp16: [p, dot*SQ + s]
        v = outp.reshape(P, IT, SQ).astype(np.float32)
        out[b, h * SQ : (h + 1) * SQ] = v.transpose(2, 1, 0).reshape(SQ, D)
    return out.reshape(B, S_, H, W)


def kernel(x, Wq, bq, Wk, bk, Wv, bv, Wo, bo, _trace=False):
    from concourse.bass_utils import run_bass_kernel_spmd

    nc = _get_nc()
    in_maps = make_in_maps(x, Wq, bq, Wk, bk, Wv, bv, Wo, bo)
    res = run_bass_kernel_spmd(nc, in_maps, list(range(N_CORES)), trace=_trace)
    out = assemble_out(res.results, x.shape)
    if _trace:
        _CACHE["last_result"] = res
    return out


# revision 19
# speedup vs baseline: 1.0134x; 1.0134x over previous
"""Trainium2 Bass kernel for nn_MultiHeadAttention_59227599012491.

Reference computation (per batch b):
    xf = x[b].reshape(S, 256)
    q  = softplus(xf @ Wq.T + bq);  k = softplus(xf @ Wk.T + bk)
    v  = xf @ Wv.T + bv
    out = ((q @ k.T) @ v) @ Wo.T + bo          (no softmax!)

No softmax -> associativity: out = q @ M + bo with
    G = k.T @ v   [256,256],   M = G @ Wo.T   [256,256]
so the S x S score matrix never exists. Sharding: B=4 batches x 2
query-halves -> 8 cores, no collectives (an NRT AllReduce of M was
measured at ~17 us fixed latency -- more than the whole dedup saves, so
k/v/G/M are recomputed by both cores of a pair; queries + output rows
are split).

Per-core pipeline (all matmuls fp16, PE computes out = lhsT.T @ rhs):
    kv loop (4 groups of 8 seq tiles): ps = x_tile @ [WkT|WvT]; DVE
        adds bk to the k plane, GpSimd adds bv to the v plane (psum ->
        fp16); batched ACT Exp+Ln softplus over each group's k planes;
        GT[d,e] += v_tile^T k_tile accumulated in PSUM across all 32
        tiles. One qT chunk is interleaved after each group so the ACT
        engine's softplus backlog hides under PE work.
    qT [e,s] = softplus(Wq x^T + bq): per-partition bias fused into the
        ACT Exp pass straight out of PSUM.
    M = G @ WoT (tiny), then outT [do,s] = M^T q^T + bo: transposed
        output so bo is per-partition (DVE tensor_scalar_add) and the
        fp16 DRAM dump is contiguous 2 KB runs per partition; the host
        un-transposes and casts back to fp32.

DMA: every input DMA moves 2 KB descriptors (host-packed layouts); each
DMA instruction occupies one HW queue (~60 GB/s at 2 KB descriptors),
so the load is split into pieces across three issuing engines (sync +
scalar HWDGE, gpsimd SWDGE) for queue parallelism, with the
first-needed pieces (Wkv, x cols 0:1024) split by partition halves to
land earliest. Output: 4 chunks of [128,1024] fp16, each written as two
[64,1024] pieces on alternating queues so the final chunk drains fast.

The activation-table pass is steered to `natural_log_exp_and_others`
(the only set holding Exp AND Ln) so the ACT engine loads its PWP
table exactly once.
"""

import numpy as np

S = 4096
SQ = 2048  # query rows per core
D = 256
P = 128
IT = D // P  # 2 contraction tiles over d
NS = S // P  # 32 sequence tiles
GRP = 8  # kv tiles per softplus batch
NG = NS // GRP
N_CORES = 8

_CACHE = {}


def _patched_act_tables(orig_fn):
    def patched(arch):
        tabs = orig_fn(arch)
        return {
            name: (s if name == "natural_log_exp_and_others" else set())
            for name, s in tabs.items()
        }

    return patched


def _build_nc():
    import concourse.bacc as bacc
    import concourse.mybir as mybir
    import concourse.tile as tile

    FP = mybir.dt.float32
    FR = mybir.dt.float16
    AF = mybir.ActivationFunctionType
    ADD = mybir.AluOpType.add

    nc = bacc.Bacc("TRN2", target_bir_lowering=False, debug=False, num_devices=1)

    # x pieces: [8, 128, 1024], piece it*4+cc = x^T[it-block, cc*1024:...]
    xp_d = nc.declare_dram_parameter("xp", [2 * 4, P, 1024], FR, isOutput=False)
    wkv_d = nc.declare_dram_parameter("wkvp", [P, 2 * 512], FR, isOutput=False)
    wqo_d = nc.declare_dram_parameter("wqop", [P, 2 * 512], FR, isOutput=False)
    bias_d = nc.declare_dram_parameter("biasp", [P, 4], FP, isOutput=False)
    bkv_d = nc.declare_dram_parameter("bkv", [1, 2 * D], FP, isOutput=False)
    outp_d = nc.declare_dram_parameter("outp", [P, 2 * SQ], FR, isOutput=True)

    def mm(psum, lhsT, rhs, start, stop):
        nc.tensor.matmul(psum, lhsT, rhs, start=start, stop=stop)

    with tile.TileContext(nc) as tc:
        with (
            tc.tile_pool(name="w", bufs=1) as wpool,
            tc.tile_pool(name="big", bufs=1) as big,
            tc.tile_pool(name="tmp", bufs=4) as tpool,
            tc.tile_pool(name="psKV", bufs=3, space="PSUM") as psKV,
            tc.tile_pool(name="psG", bufs=1, space="PSUM") as psG,
            tc.tile_pool(name="psQ", bufs=3, space="PSUM") as psQ,
        ):
            # SBUF weight layout: cols [wkv 512 | wq 256 | wo 256] per it
            w_sb = wpool.tile([P, IT, 1024], FR, tag="w")
            xbT_sb = big.tile([P, IT, S], FR, tag="xbT")
            bias_sb = wpool.tile([P, 4], FP, tag="bias")
            bkv_bc = wpool.tile([P, 2 * D], FP, tag="bkv")
            kv_sb = big.tile([P, 2, NS, D], FR, tag="kv")
            qT_sb = big.tile([P, IT, SQ], FR, tag="qT")
            GT_sb = wpool.tile([P, IT, D], FR, tag="GT")
            M_sb = wpool.tile([P, IT, D], FR, tag="M")
            outT_sb = big.tile([P, IT, SQ], FR, tag="outT")

            # --- input DMAs: one HW queue per instruction, spread over
            # sync/scalar (HWDGE) + gpsimd (SWDGE); first-needed first ---
            nc.sync.dma_start(
                w_sb[0:64, :, 0:512],
                wkv_d.ap()[0:64, :].rearrange("p (i w) -> p i w", i=IT),
            )
            nc.scalar.dma_start(
                w_sb[64:P, :, 0:512],
                wkv_d.ap()[64:P, :].rearrange("p (i w) -> p i w", i=IT),
            )
            nc.gpsimd.dma_start(
                bkv_bc[:, :], bkv_d.ap()[0:1, :].broadcast_to([P, 2 * D])
            )
            nc.gpsimd.dma_start(bias_sb[:, :], bias_d.ap()[:, :])

            def xpiece(eng, it, cc, hp=slice(0, P)):
                eng.dma_start(
                    xbT_sb[hp, it, cc * 1024 : (cc + 1) * 1024],
                    xp_d.ap()[it * 4 + cc, hp, :],
                )

            xpiece(nc.sync, 0, 0)
            xpiece(nc.scalar, 1, 0)
            xpiece(nc.sync, 0, 1)
            xpiece(nc.gpsimd, 1, 1)
            xpiece(nc.scalar, 0, 2)
            xpiece(nc.sync, 1, 2)
            xpiece(nc.scalar, 0, 3)
            xpiece(nc.sync, 1, 3)
            nc.gpsimd.dma_start(
                w_sb[:, :, 512:1024],
                wqo_d.ap()[:, :].rearrange("p (i w) -> p i w", i=IT),
            )

            wkv = w_sb[:, :, 0:512]

            # --- kv = x [WkT | WvT] + [bk | bv]; softplus k in 4-tile runs ---
            for t in range(NS):
                ts = slice(t * P, (t + 1) * P)
                ps = psKV.tile([P, 2 * D], FP, tag="psKV")
                for it in range(IT):
                    mm(ps[:, :], xbT_sb[:, it, ts], wkv[:, it, :], it == 0, it == IT - 1)
                nc.vector.tensor_tensor(
                    kv_sb[:, :, t, :],
                    ps[:, :].rearrange("p (j d) -> p j d", j=2),
                    bkv_bc[:, :].rearrange("p (j d) -> p j d", j=2),
                    op=ADD,
                )
                if t % 4 == 3:
                    tt = slice(t - 3, t + 1)
                    tmpk = tpool.tile([P, 4, D], FP, tag="tmpk")
                    nc.scalar.activation(tmpk[:, :, :], kv_sb[:, 0, tt, :], AF.Exp)
                    nc.scalar.activation(kv_sb[:, 0, tt, :], tmpk[:, :, :], AF.Ln, bias=1.0)

            # --- qT = softplus(Wq x^T + bq), [e, s]; bias fused in Exp ---
            for dt in range(IT):
                for blk in range(SQ // 512):
                    ss = slice(blk * 512, (blk + 1) * 512)
                    ps = psQ.tile([P, 512], FP, tag="psQ")
                    for it in range(IT):
                        mm(
                            ps[:, :],
                            w_sb[:, it, 512 + dt * P : 512 + (dt + 1) * P],
                            xbT_sb[:, it, ss],
                            it == 0,
                            it == IT - 1,
                        )
                    tmpq = tpool.tile([P, 512], FP, tag="tmpq")
                    nc.scalar.activation(
                        tmpq[:, :], ps[:, :], AF.Exp, bias=bias_sb[:, dt : dt + 1]
                    )
                    nc.scalar.activation(qT_sb[:, dt, ss], tmpq[:, :], AF.Ln, bias=1.0)

            # --- GT[d, e] = sum_s v[s, d] k[s, e] (pure PE streaming) ---
            GTps = []
            for dt in range(IT):
                gt = psG.tile([P, D], FP, tag=f"psG{dt}", name=f"GTps{dt}")
                GTps.append(gt)
            for dt in range(IT):
                vs = slice(dt * P, (dt + 1) * P)
                for t in range(NS):
                    mm(GTps[dt], kv_sb[:, 1, t, vs], kv_sb[:, 0, t, :], t == 0, t == NS - 1)
                nc.vector.tensor_copy(GT_sb[:, dt, :], GTps[dt][:, :])

            # --- M = GT^T @ WoT ---
            for et in range(IT):
                ps = psQ.tile([P, 512], FP, tag="psQ")
                for dt in range(IT):
                    mm(
                        ps[:, 0:D],
                        GT_sb[:, dt, et * P : (et + 1) * P],
                        w_sb[:, dt, 768:1024],
                        dt == 0,
                        dt == IT - 1,
                    )
                nc.vector.tensor_copy(M_sb[:, et, :], ps[:, 0:D])

            # --- outT[do, s] = M^T q^T + bo (per-partition bias, fp16) ---
            for dot in range(IT):
                for blk in range(SQ // 512):
                    ss = slice(blk * 512, (blk + 1) * 512)
                    ps = psQ.tile([P, 512], FP, tag="psQ")
                    for et in range(IT):
                        mm(
                            ps[:, :],
                            M_sb[:, et, dot * P : (dot + 1) * P],
                            qT_sb[:, et, ss],
                            et == 0,
                            et == IT - 1,
                        )
                    nc.vector.tensor_scalar_add(
                        outT_sb[:, dot, ss], ps[:, :], bias_sb[:, 2 + dot : 3 + dot]
                    )
                    if blk % 2 == 1:
                        off = dot * SQ + (blk - 1) * 512
                        src = outT_sb[:, dot, (blk - 1) * 512 : (blk + 1) * 512]
                        last = dot == IT - 1 and blk == SQ // 512 - 1
                        if last:
                            engs = [nc.sync, nc.gpsimd, nc.scalar, nc.sync]
                            for qi in range(4):
                                hp = slice(qi * 32, (qi + 1) * 32)
                                engs[qi].dma_start(
                                    outp_d.ap()[hp, off : off + 1024], src[hp, :]
                                )
                        else:
                            nc.sync.dma_start(outp_d.ap()[:, off : off + 1024], src)

    import concourse.hw_specs as hw_specs

    orig = bacc.get_activation_tables
    bacc.get_activation_tables = _patched_act_tables(hw_specs.get_activation_tables)
    try:
        nc.compile()
    finally:
        bacc.get_activation_tables = orig
    return nc


def _get_nc():
    nc = _CACHE.get("nc")
    if nc is None:
        nc = _build_nc()
        _CACHE["nc"] = nc
    return nc


def make_in_maps(x, Wq, bq, Wk, bk, Wv, bv, Wo, bo):
    B = x.shape[0]
    f16 = np.float16
    xf = np.asarray(x, dtype=np.float32).reshape(B, S, D)
    xfT = np.ascontiguousarray(xf.transpose(0, 2, 1).astype(f16))  # [B, 256, 4096]

    def pack_it(wT):  # [256, C] -> [128, 2*C] with it-blocks side by side
        C = wT.shape[1]
        return np.ascontiguousarray(
            wT.reshape(IT, P, C).transpose(1, 0, 2).reshape(P, IT * C)
        )

    wkvp = pack_it(np.hstack([np.asarray(Wk, f16).T, np.asarray(Wv, f16).T]))
    wqop = pack_it(np.hstack([np.asarray(Wq, f16).T, np.asarray(Wo, f16).T]))
    biasp = np.stack(
        [
            np.asarray(bq, np.float32)[0:P],
            np.asarray(bq, np.float32)[P:D],
            np.asarray(bo, np.float32)[0:P],
            np.asarray(bo, np.float32)[P:D],
        ],
        axis=1,
    )
    shared = {
        "wkvp": wkvp,
        "wqop": wqop,
        "biasp": np.ascontiguousarray(biasp),
        "bkv": np.concatenate(
            [np.asarray(bk, np.float32), np.asarray(bv, np.float32)]
        ).reshape(1, 2 * D),
    }
    in_maps = []
    for c in range(N_CORES):
        b, h = divmod(c, 2)
        xT = xfT[b]
        if h == 1:
            xT = np.concatenate([xT[:, SQ:], xT[:, :SQ]], axis=1)
        # pieces [it*4+cc] = [128, 1024]
        xpc = np.ascontiguousarray(
            xT.reshape(IT, P, 4, 1024).transpose(0, 2, 1, 3).reshape(8, P, 1024)
        )
        in_maps.append({"xp": xpc, **shared})
    return in_maps


def assemble_out(results, x_shape):
    B, S_, H, W = x_shape
    out = np.empty((B, S_, D), np.float32)
    for c in range(N_CORES):
        b, h = divmod(c, 2)
        outp = results[c]["outp"]  # [128, 2*SQ] fp16: [p, dot*SQ + s]
        v = outp.reshape(P, IT, SQ).astype(np.float32)
        out[b, h * SQ : (h + 1) * SQ] = v.transpose(2, 1, 0).reshape(SQ, D)
    return out.reshape(B, S_, H, W)


def kernel(x, Wq, bq, Wk, bk, Wv, bv, Wo, bo, _trace=False):
    from concourse.bass_utils import run_bass_kernel_spmd

    nc = _get_nc()
    in_maps = make_in_maps(x, Wq, bq, Wk, bk, Wv, bv, Wo, bo)
    res = run_bass_kernel_spmd(nc, in_maps, list(range(N_CORES)), trace=_trace)
    out = assemble_out(res.results, x.shape)
    if _trace:
        _CACHE["last_result"] = res
    return out


# revision 20
# speedup vs baseline: 1.0314x; 1.0178x over previous
"""Trainium2 Bass kernel for nn_MultiHeadAttention_59227599012491.

Reference computation (per batch b):
    xf = x[b].reshape(S, 256)
    q  = softplus(xf @ Wq.T + bq);  k = softplus(xf @ Wk.T + bk)
    v  = xf @ Wv.T + bv
    out = ((q @ k.T) @ v) @ Wo.T + bo          (no softmax!)

No softmax -> associativity: out = q @ M + bo with
    G = k.T @ v   [256,256],   M = G @ Wo.T   [256,256]
so the S x S score matrix never exists. Sharding: B=4 batches x 2
query-halves -> 8 cores, no collectives (an NRT AllReduce of M was
measured at ~17 us fixed latency -- more than the whole dedup saves, so
k/v/G/M are recomputed by both cores of a pair; queries + output rows
are split).

Per-core pipeline (all matmuls fp16, PE computes out = lhsT.T @ rhs):
    kv loop (4 groups of 8 seq tiles): ps = x_tile @ [WkT|WvT]; DVE
        adds bk to the k plane, GpSimd adds bv to the v plane (psum ->
        fp16); batched ACT Exp+Ln softplus over each group's k planes;
        GT[d,e] += v_tile^T k_tile accumulated in PSUM across all 32
        tiles. One qT chunk is interleaved after each group so the ACT
        engine's softplus backlog hides under PE work.
    qT [e,s] = softplus(Wq x^T + bq): per-partition bias fused into the
        ACT Exp pass straight out of PSUM.
    M = G @ WoT (tiny), then outT [do,s] = M^T q^T + bo: transposed
        output so bo is per-partition (DVE tensor_scalar_add) and the
        fp16 DRAM dump is contiguous 2 KB runs per partition; the host
        un-transposes and casts back to fp32.

DMA: every input DMA moves 2 KB descriptors (host-packed layouts); each
DMA instruction occupies one HW queue (~60 GB/s at 2 KB descriptors),
so the load is split into pieces across three issuing engines (sync +
scalar HWDGE, gpsimd SWDGE) for queue parallelism, with the
first-needed pieces (Wkv, x cols 0:1024) split by partition halves to
land earliest. Output: 4 chunks of [128,1024] fp16, each written as two
[64,1024] pieces on alternating queues so the final chunk drains fast.

The activation-table pass is steered to `natural_log_exp_and_others`
(the only set holding Exp AND Ln) so the ACT engine loads its PWP
table exactly once.
"""

import numpy as np

S = 4096
SQ = 2048  # query rows per core
D = 256
P = 128
IT = D // P  # 2 contraction tiles over d
NS = S // P  # 32 sequence tiles
GRP = 8  # kv tiles per softplus batch
NG = NS // GRP
N_CORES = 8

_CACHE = {}


def _patched_act_tables(orig_fn):
    def patched(arch):
        tabs = orig_fn(arch)
        return {
            name: (s if name == "natural_log_exp_and_others" else set())
            for name, s in tabs.items()
        }

    return patched


def _build_nc():
    import concourse.bacc as bacc
    import concourse.mybir as mybir
    import concourse.tile as tile

    FP = mybir.dt.float32
    FR = mybir.dt.float16
    AF = mybir.ActivationFunctionType
    ADD = mybir.AluOpType.add

    nc = bacc.Bacc("TRN2", target_bir_lowering=False, debug=False, num_devices=1)

    # x pieces: [8, 128, 1024], piece it*4+cc = x^T[it-block, cc*1024:...]
    xp_d = nc.declare_dram_parameter("xp", [2 * 4, P, 1024], FR, isOutput=False)
    wkv_d = nc.declare_dram_parameter("wkvp", [P, 2 * 512], FR, isOutput=False)
    wqo_d = nc.declare_dram_parameter("wqop", [P, 2 * 512], FR, isOutput=False)
    bias_d = nc.declare_dram_parameter("biasp", [P, 4], FP, isOutput=False)
    bkv_d = nc.declare_dram_parameter("bkv", [1, 2 * D], FP, isOutput=False)
    outp_d = nc.declare_dram_parameter("outp", [P, 2 * SQ], FR, isOutput=True)

    def mm(psum, lhsT, rhs, start, stop):
        nc.tensor.matmul(psum, lhsT, rhs, start=start, stop=stop)

    with tile.TileContext(nc) as tc:
        with (
            tc.tile_pool(name="w", bufs=1) as wpool,
            tc.tile_pool(name="big", bufs=1) as big,
            tc.tile_pool(name="tmp", bufs=4) as tpool,
            tc.tile_pool(name="psKV", bufs=3, space="PSUM") as psKV,
            tc.tile_pool(name="psG", bufs=1, space="PSUM") as psG,
            tc.tile_pool(name="psQ", bufs=3, space="PSUM") as psQ,
        ):
            # SBUF weight layout: cols [wkv 512 | wq 256 | wo 256] per it
            w_sb = wpool.tile([P, IT, 1024], FR, tag="w")
            xbT_sb = big.tile([P, IT, S], FR, tag="xbT")
            bias_sb = wpool.tile([P, 4], FP, tag="bias")
            bkv_bc = wpool.tile([P, 2 * D], FP, tag="bkv")
            kv_sb = big.tile([P, 2, NS, D], FR, tag="kv")
            qT_sb = big.tile([P, IT, SQ], FR, tag="qT")
            GT_sb = wpool.tile([P, IT, D], FR, tag="GT")
            M_sb = wpool.tile([P, IT, D], FR, tag="M")
            outT_sb = big.tile([P, IT, SQ], FR, tag="outT")

            # --- input DMAs: one HW queue per instruction, spread over
            # sync/scalar (HWDGE) + gpsimd (SWDGE); first-needed first ---
            nc.sync.dma_start(
                w_sb[0:64, :, 0:512],
                wkv_d.ap()[0:64, :].rearrange("p (i w) -> p i w", i=IT),
            )
            nc.scalar.dma_start(
                w_sb[64:P, :, 0:512],
                wkv_d.ap()[64:P, :].rearrange("p (i w) -> p i w", i=IT),
            )
            nc.gpsimd.dma_start(
                bkv_bc[:, :], bkv_d.ap()[0:1, :].broadcast_to([P, 2 * D])
            )
            nc.gpsimd.dma_start(bias_sb[:, :], bias_d.ap()[:, :])

            def xpiece(eng, it, cc, hp=slice(0, P)):
                eng.dma_start(
                    xbT_sb[hp, it, cc * 1024 : (cc + 1) * 1024],
                    xp_d.ap()[it * 4 + cc, hp, :],
                )

            xpiece(nc.sync, 0, 0)
            xpiece(nc.scalar, 1, 0)
            xpiece(nc.sync, 0, 1)
            xpiece(nc.gpsimd, 1, 1)
            xpiece(nc.scalar, 0, 2)
            xpiece(nc.sync, 1, 2)
            xpiece(nc.scalar, 0, 3)
            xpiece(nc.sync, 1, 3)
            nc.gpsimd.dma_start(
                w_sb[:, :, 512:1024],
                wqo_d.ap()[:, :].rearrange("p (i w) -> p i w", i=IT),
            )

            wkv = w_sb[:, :, 0:512]

            # --- kv = x [WkT | WvT] + [bk | bv]; softplus k in 4-tile runs ---
            for t in range(NS):
                ts = slice(t * P, (t + 1) * P)
                ps = psKV.tile([P, 2 * D], FP, tag="psKV")
                for it in range(IT):
                    mm(ps[:, :], xbT_sb[:, it, ts], wkv[:, it, :], it == 0, it == IT - 1)
                nc.vector.tensor_tensor(
                    kv_sb[:, :, t, :],
                    ps[:, :].rearrange("p (j d) -> p j d", j=2),
                    bkv_bc[:, :].rearrange("p (j d) -> p j d", j=2),
                    op=ADD,
                )
                if t % 4 == 3:
                    tt = slice(t - 3, t + 1)
                    tmpk = tpool.tile([P, 4, D], FP, tag="tmpk")
                    nc.scalar.activation(tmpk[:, :, :], kv_sb[:, 0, tt, :], AF.Exp)
                    nc.scalar.activation(kv_sb[:, 0, tt, :], tmpk[:, :, :], AF.Ln, bias=1.0)

            # --- qT = softplus(Wq x^T + bq), [e, s]; bias fused in Exp ---
            for dt in range(IT):
                for blk in range(SQ // 512):
                    ss = slice(blk * 512, (blk + 1) * 512)
                    ps = psQ.tile([P, 512], FP, tag="psQ")
                    for it in range(IT):
                        mm(
                            ps[:, :],
                            w_sb[:, it, 512 + dt * P : 512 + (dt + 1) * P],
                            xbT_sb[:, it, ss],
                            it == 0,
                            it == IT - 1,
                        )
                    tmpq = tpool.tile([P, 512], FP, tag="tmpq")
                    nc.scalar.activation(
                        tmpq[:, :], ps[:, :], AF.Exp, bias=bias_sb[:, dt : dt + 1]
                    )
                    nc.scalar.activation(qT_sb[:, dt, ss], tmpq[:, :], AF.Ln, bias=1.0)

            # --- GT[d, e] = sum_s v[s, d] k[s, e] (pure PE streaming) ---
            GTps = []
            for dt in range(IT):
                gt = psG.tile([P, D], FP, tag=f"psG{dt}", name=f"GTps{dt}")
                GTps.append(gt)
            for dt in range(IT):
                vs = slice(dt * P, (dt + 1) * P)
                for t in range(NS):
                    mm(GTps[dt], kv_sb[:, 1, t, vs], kv_sb[:, 0, t, :], t == 0, t == NS - 1)
                nc.vector.tensor_copy(GT_sb[:, dt, :], GTps[dt][:, :])

            # --- M = GT^T @ WoT ---
            for et in range(IT):
                ps = psQ.tile([P, 512], FP, tag="psQ")
                for dt in range(IT):
                    mm(
                        ps[:, 0:D],
                        GT_sb[:, dt, et * P : (et + 1) * P],
                        w_sb[:, dt, 768:1024],
                        dt == 0,
                        dt == IT - 1,
                    )
                nc.vector.tensor_copy(M_sb[:, et, :], ps[:, 0:D])

            # --- outT[do, s] = M^T q^T + bo (per-partition bias, fp16) ---
            for dot in range(IT):
                for blk in range(SQ // 512):
                    ss = slice(blk * 512, (blk + 1) * 512)
                    ps = psQ.tile([P, 512], FP, tag="psQ")
                    for et in range(IT):
                        mm(
                            ps[:, :],
                            M_sb[:, et, dot * P : (dot + 1) * P],
                            qT_sb[:, et, ss],
                            et == 0,
                            et == IT - 1,
                        )
                    nc.vector.tensor_scalar_add(
                        outT_sb[:, dot, ss], ps[:, :], bias_sb[:, 2 + dot : 3 + dot]
                    )
                    if blk % 2 == 1:
                        off = dot * SQ + (blk - 1) * 512
                        src = outT_sb[:, dot, (blk - 1) * 512 : (blk + 1) * 512]
                        last = dot == IT - 1 and blk == SQ // 512 - 1
                        if last:
                            nc.sync.dma_start(
                                outp_d.ap()[0:64, off : off + 1024], src[0:64, :]
                            )
                            nc.gpsimd.dma_start(
                                outp_d.ap()[64:P, off : off + 1024], src[64:P, :]
                            )
                        else:
                            nc.sync.dma_start(outp_d.ap()[:, off : off + 1024], src)

    import concourse.hw_specs as hw_specs

    orig = bacc.get_activation_tables
    bacc.get_activation_tables = _patched_act_tables(hw_specs.get_activation_tables)
    try:
        nc.compile()
    finally:
        bacc.get_activation_tables = orig
    return nc


def _get_nc():
    nc = _CACHE.get("nc")
    if nc is None:
        nc = _build_nc()
        _CACHE["nc"] = nc
    return nc


def make_in_maps(x, Wq, bq, Wk, bk, Wv, bv, Wo, bo):
    B = x.shape[0]
    f16 = np.float16
    xf = np.asarray(x, dtype=np.float32).reshape(B, S, D)
    xfT = np.ascontiguousarray(xf.transpose(0, 2, 1).astype(f16))  # [B, 256, 4096]

    def pack_it(wT):  # [256, C] -> [128, 2*C] with it-blocks side by side
        C = wT.shape[1]
        return np.ascontiguousarray(
            wT.reshape(IT, P, C).transpose(1, 0, 2).reshape(P, IT * C)
        )

    wkvp = pack_it(np.hstack([np.asarray(Wk, f16).T, np.asarray(Wv, f16).T]))
    wqop = pack_it(np.hstack([np.asarray(Wq, f16).T, np.asarray(Wo, f16).T]))
    biasp = np.stack(
        [
            np.asarray(bq, np.float32)[0:P],
            np.asarray(bq, np.float32)[P:D],
            np.asarray(bo, np.float32)[0:P],
            np.asarray(bo, np.float32)[P:D],
        ],
        axis=1,
    )
    shared = {
        "wkvp": wkvp,
        "wqop": wqop,
        "biasp": np.ascontiguousarray(biasp),
        "bkv": np.concatenate(
            [np.asarray(bk, np.float32), np.asarray(bv, np.float32)]
        ).reshape(1, 2 * D),
    }
    in_maps = []
    for c in range(N_CORES):
        b, h = divmod(c, 2)
        xT = xfT[b]
        if h == 1:
            xT = np.concatenate([xT[:, SQ:], xT[:, :SQ]], axis=1)
        # pieces [it*4+cc] = [128, 1024]
        xpc = np.ascontiguousarray(
            xT.reshape(IT, P, 4, 1024).transpose(0, 2, 1, 3).reshape(8, P, 1024)
        )
        in_maps.append({"xp": xpc, **shared})
    return in_maps


def assemble_out(results, x_shape):
    B, S_, H, W = x_shape
    out = np.empty((B, S_, D), np.float32)
    for c in range(N_CORES):
        b, h = divmod(c, 2)
        outp = results[c]["outp"]  # [128, 2*SQ] fp16: [p, dot*SQ + s]
        v = outp.reshape(P, IT, SQ).astype(np.float32)
        out[b, h * SQ : (h + 1) * SQ] = v.transpose(2, 1, 0).reshape(SQ, D)
    return out.reshape(B, S_, H, W)


def kernel(x, Wq, bq, Wk, bk, Wv, bv, Wo, bo, _trace=False):
    from concourse.bass_utils import run_bass_kernel_spmd

    nc = _get_nc()
    in_maps = make_in_maps(x, Wq, bq, Wk, bk, Wv, bv, Wo, bo)
    res = run_bass_kernel_spmd(nc, in_maps, list(range(N_CORES)), trace=_trace)
    out = assemble_out(res.results, x.shape)
    if _trace:
        _CACHE["last_result"] = res
    return out
